# revision 13
# baseline (speedup 1.0000x reference)
"""CodebookLinear TRN2 kernel.

Reference computation (jax):
    W = codebook[indices].reshape(-1)[:4096*4096].reshape(4096, 4096)   # [out, in]
    out = einsum('bsi,oi->bso', x, W) + bias

Distribution: 8 NeuronCores in a 4 (out_features) x 2 (tokens) grid, no
collectives. Each core:

  setup:   PE-transposes the codebook to cb^T and lays it out so SBUF
           partition p holds codebook column k(p)  -> data[128, 4096].
  phase 1: reconstructs its W^T shard [i, o] in bf16 directly in SBUF via
           the Pool engine's hardware indirect gather (InstIndirectCopy):
           per 16-partition group the index list interleaves the two
           j-block columns owned by that group; a stride-2-free
           tensor_copy + copy_predicated (per-partition parity mask)
           selects/casts into the resident W^T.
  phase 2: streams x^T tiles (f32->bf16 cast during DMA), accumulates
           out[t, o] = x^T[:,t].T @ W^T[:,o] in PSUM over 32 k-tiles
           (PSUM preloaded with bias via a K=1 f32 matmul), copies
           PSUM->SBUF and DMAs out rows.

Host side only shards/reshapes: x is passed transposed and row-permuted
(layout choice), indices are converted to uint16 and pre-permuted into
the wrapped per-group interleaved layout the gather consumes (pure
permutation), bias is sliced.  Output is assembled to [4, 2048, 4096].

Index/partition math (per core, o local in [0, O_LOC)):
  Within k-tile it, SBUF partition p holds contraction row
      i = 128*it + sigma(p),  sigma(p) = 8*(2*(p>>4) + (p&1)) + ((p>>1)&7)
  so  j(i) = 16*it + 2*g + h,  k(i) = (p>>1)&7,  g = p>>4,  h = p&1.
  group g's list for k-tile it:  L[n = 2*o + h] = idx[o, 16*it + 2*g + h]
  wrapped storage:               idxw[16*g + q, it, f] = L[16*f + q]
  gather:  g2[p, n] = data[p, L[g(p)][n]] = cb[idx[o(n), j], k(p)]
  select:  W^T[p, o] = g2[p, 2*o + (p&1)]
"""

import sys

for _p in ("/opt/trn_rl_repo",):
    if _p not in sys.path:
        sys.path.insert(0, _p)

import numpy as np

import concourse.bacc as bacc
import concourse.mybir as mybir
import concourse.tile as tile
from concourse.bass_utils import run_bass_kernel_spmd
from concourse.masks import make_identity

# Problem constants
OUT_F = 4096
IN_F = 4096
KCB = 4096          # codebook entries
BS = 8              # block size
JB = IN_F // BS     # 512 blocks per W row
B, S = 4, 2048
T = B * S           # 8192 tokens

# Shard grid: S_O x S_T = 8 cores
S_O, S_T = 8, 1
O_LOC = OUT_F // S_O   # 1024
T_LOC = T // S_T       # 4096

P = 128
NIT = IN_F // P        # 32 k-tiles
NTT = T_LOC // P       # 32 token tiles
NOH = 1                # whole o-shard fits one PSUM pass (512)

# partition -> within-tile contraction row
_p_ar = np.arange(P)
SIGMA = (8 * (2 * (_p_ar >> 4) + (_p_ar & 1)) + ((_p_ar >> 1) & 7)).astype(np.int64)

_nc_cache = None
last_result = None     # BassKernelResults of the most recent run (for test.py)


def build_nc():
    nc = bacc.Bacc("TRN2", target_bir_lowering=False, debug=False)
    xT = nc.dram_tensor("xT", [IN_F, T_LOC], mybir.dt.float32, kind="ExternalInput")
    idxw = nc.dram_tensor("idxw", [P, NIT * (2 * O_LOC // 16)], mybir.dt.int16, kind="ExternalInput")
    cb = nc.dram_tensor("cb", [KCB, BS], mybir.dt.float32, kind="ExternalInput")
    bias = nc.dram_tensor("bias", [1, O_LOC], mybir.dt.float32, kind="ExternalInput")
    mask = nc.dram_tensor("mask", [P, 1], mybir.dt.uint8, kind="ExternalInput")
    out = nc.dram_tensor("out", [T_LOC, O_LOC], mybir.dt.float32, kind="ExternalOutput")
    cbt16_dram = nc.dram_tensor("cbt16_scratch", [16, KCB], mybir.dt.float32)

    with tile.TileContext(nc) as tc:
        with (
            tc.tile_pool(name="const", bufs=1) as constp,
            tc.tile_pool(name="wt", bufs=1) as wtp,
            tc.tile_pool(name="g2p", bufs=2) as g2p,
            tc.tile_pool(name="xp", bufs=3) as xp,
            tc.tile_pool(name="xbp", bufs=3) as xbp,
            tc.tile_pool(name="outp", bufs=2) as outp,
            tc.tile_pool(name="psmm", bufs=4, space="PSUM") as psmm,
            tc.tile_pool(name="pstr", bufs=2, space="PSUM") as pstr,
        ):
            identity = constp.tile([P, P], mybir.dt.float32)
            make_identity(nc, identity[:])
            ones_row = constp.tile([1, P], mybir.dt.float32)
            nc.gpsimd.memset(ones_row[:], 1.0)
            bias_row = constp.tile([1, O_LOC], mybir.dt.float32)
            nc.sync.dma_start(out=bias_row[:], in_=bias[:, :])
            mask_t = constp.tile([P, 1], mybir.dt.uint8)
            nc.sync.dma_start(out=mask_t[:], in_=mask[:, :])

            # ---- setup: cb^T, duplicated pairwise, replicated to 128 parts ----
            cbn = constp.tile([P, NIT * BS], mybir.dt.float32)  # cb rows on partitions
            nc.sync.dma_start(
                out=cbn[:].rearrange("p (a b) -> p a b", b=BS),
                in_=cb[:, :].rearrange("(a p) b -> p a b", p=P),
            )
            cbt = constp.tile([BS, KCB], mybir.dt.float32)
            for a in range(NIT):
                pst = pstr.tile([BS, P], mybir.dt.float32)
                nc.tensor.transpose(
                    out=pst[:],
                    in_=cbn[:, a * BS : (a + 1) * BS],
                    identity=identity[:],
                )
                nc.vector.tensor_copy(out=cbt[:, a * P : (a + 1) * P], in_=pst[:])
            # cbt16[2k + h] = cbt[k]
            for h in range(2):
                nc.sync.dma_start(
                    out=cbt16_dram[:, :].rearrange("(a h) f -> a h f", h=2)[:, h],
                    in_=cbt[:],
                )
            data = constp.tile([P, KCB], mybir.dt.float32)
            for g in range(8):
                nc.sync.dma_start(
                    out=data[16 * g : 16 * (g + 1), :], in_=cbt16_dram[:, :]
                )

            # indices, pre-wrapped on host
            idxt = constp.tile([P, NIT * (2 * O_LOC // 16)], mybir.dt.int16)
            nc.sync.dma_start(out=idxt[:], in_=idxw[:, :])

            # Resident W^T, bf16: [p, k-tile, o]
            WT = wtp.tile([P, NIT, O_LOC], mybir.dt.bfloat16)

            # ---- phase 1: Q7 gather (ap_gather) + parity select ----
            FW = 2 * O_LOC // 16   # wrapped index columns per k-tile
            mask_bc = mask_t[:, 0:1].to_broadcast([P, O_LOC])
            for it in range(NIT):
                g2 = g2p.tile([P, 2 * O_LOC], mybir.dt.float32)
                nc.gpsimd.ap_gather(
                    out_ap=g2[:, :],
                    in_ap=data[:, :],
                    idxs_ap=idxt[:, it * FW : (it + 1) * FW],
                    channels=P,
                    num_elems=KCB,
                    d=1,
                    num_idxs=2 * O_LOC,
                )
                g2_s = g2[:, :].rearrange("p (o s) -> p o s", s=2)
                nc.vector.tensor_copy(out=WT[:, it, :], in_=g2_s[:, :, 0])
                nc.vector.copy_predicated(
                    out=WT[:, it, :], mask=mask_bc, data=g2_s[:, :, 1]
                )

            # ---- phase 2: stream x^T, matmul, bias, store ----
            xTr = xT[:, :].rearrange("(it p) t -> p it t", p=P)  # [128, NIT, T_LOC]
            for tt in range(NTT):
                # HWDGE f32 load (SP engine: prefetches during the Pool gathers),
                # then cast to bf16 on DVE/ACT (alternating to spread load)
                xf = xp.tile([P, NIT, P], mybir.dt.float32, name="xf")
                nc.sync.dma_start(out=xf[:, :, :], in_=xTr[:, :, tt * P : (tt + 1) * P])
                xt = xbp.tile([P, NIT, P], mybir.dt.bfloat16)
                if tt % 2 == 0:
                    nc.vector.tensor_copy(out=xt[:, :, :], in_=xf[:, :, :])
                else:
                    nc.scalar.copy(out=xt[:, :, :], in_=xf[:, :, :])
                outt = outp.tile([P, O_LOC], mybir.dt.float32)
                ps = psmm.tile([P, O_LOC], mybir.dt.float32)
                # bias preload: psum[t, o] = ones[t] * bias[o]
                nc.tensor.matmul(
                    out=ps[:],
                    lhsT=ones_row[:, :],
                    rhs=bias_row[:, :],
                    start=True,
                    stop=False,
                )
                for it in range(NIT):
                    nc.tensor.matmul(
                        out=ps[:],
                        lhsT=xt[:, it, :],
                        rhs=WT[:, it, :],
                        start=False,
                        stop=(it == NIT - 1),
                    )
                nc.vector.tensor_copy(out=outt[:, :], in_=ps[:])
                nc.sync.dma_start(out=out[tt * P : (tt + 1) * P, :], in_=outt[:])

    nc.compile()
    return nc


def _get_nc():
    global _nc_cache
    if _nc_cache is None:
        _nc_cache = build_nc()
    return _nc_cache


def _wrap_indices(idx_local):
    """[O_LOC, JB] int -> wrapped interleaved uint16 [P, NIT*P]."""
    arr = idx_local.reshape(O_LOC, NIT, 8, 2)        # [o, it, g, h]
    L = arr.transpose(2, 1, 0, 3).reshape(8, NIT, 2 * O_LOC)   # [g, it, n=2o+h]
    Lw = L.reshape(8, NIT, 2 * O_LOC // 16, 16)      # [g, it, f, q]
    idxw = Lw.transpose(0, 3, 1, 2).reshape(P, NIT * (2 * O_LOC // 16))
    return np.ascontiguousarray(idxw.astype(np.int16))


def make_in_maps(x, codebook, indices, bias):
    x = np.asarray(x, dtype=np.float32).reshape(T, IN_F)
    xT_full = np.ascontiguousarray(x.T)  # [IN_F, T]
    # permute contraction rows within each 128-tile to match the W^T layout
    xT_perm = np.ascontiguousarray(
        xT_full.reshape(NIT, P, T)[:, SIGMA, :].reshape(IN_F, T)
    )
    idx2d = np.asarray(indices).astype(np.int64).reshape(OUT_F, JB)
    cb = np.ascontiguousarray(np.asarray(codebook, dtype=np.float32))
    b = np.asarray(bias, dtype=np.float32)
    mask_np = (np.arange(P) % 2).astype(np.uint8).reshape(P, 1)

    in_maps = []
    for c in range(8):
        ot, tt = c % S_O, c // S_O
        in_maps.append(
            {
                "xT": np.ascontiguousarray(xT_perm[:, tt * T_LOC : (tt + 1) * T_LOC]),
                "idxw": _wrap_indices(idx2d[ot * O_LOC : (ot + 1) * O_LOC]),
                "cb": cb,
                "bias": np.ascontiguousarray(
                    b[ot * O_LOC : (ot + 1) * O_LOC]
                ).reshape(1, O_LOC),
                "mask": mask_np,
            }
        )
    return in_maps


def assemble(outs):
    full = np.empty((T, OUT_F), dtype=np.float32)
    for c in range(8):
        ot, tt = c % S_O, c // S_O
        full[tt * T_LOC : (tt + 1) * T_LOC, ot * O_LOC : (ot + 1) * O_LOC] = outs[c][
            "out"
        ]
    return full.reshape(B, S, OUT_F)


def kernel(x, codebook, indices, bias):
    global last_result
    nc = _get_nc()
    in_maps = make_in_maps(x, codebook, indices, bias)
    last_result = run_bass_kernel_spmd(nc, in_maps, core_ids=list(range(8)))
    return assemble(last_result.results)
